# revision 51
# baseline (speedup 1.0000x reference)
"""Trainium2 Bass kernel for nn_DinoGazeSpade (segment_reduce + SPADE stack).

Strategy (8 NeuronCores, SPMD single program):
  - Two groups of 4 cores; group = batch index b (0..1), rank g = core % 4.
  - Painted-map + bilinear resize is reformulated as segment matrices:
        sem_rs[c,p,q] = sum_s avg[s,c] * M[s,p,q],
        M[s] = R @ onehot_s @ C^T   (R, C: 32x448 separable resize matrices)
    Each core builds avg + M for its 16 segments only, then ONE small
    in-group AllGather ships (avg ‖ M ‖ LN0 stats) [17,1408] bf16 (~48KB);
    every core then computes the FULL sem_rs with a cheap k=64 matmul set.
  - Shared 3x3 convs: every rank computes L0 locally; rank pairs (0,1)/(2,3)
    compute the full L1/L2 and an AllGather (hidden under the L0 gamma/beta
    convs) distributes them.
  - gamma/beta convs of layer 0 are split by output channel (384 per core),
    run in fp8 DoubleRow mode (2 taps per matmul stream); the pointwise c0
    conv partials are AllReduced in bf16 (C2), overlapped by the prefired
    L1/L2 conv matmuls.  Layers 1-2 are tiny and replicated.
  - conv3x3 = 9 shifted matmuls over a zero-padded [C,34,34] SBUF image.
  - LayerNorm scalars use a ones/N-matmul reduce+broadcast (no DRAM
    roundtrip); softplus = ln(1+exp(t)) in two ACT ops sharing one table
    set, with the LN sum folded into the Ln via accum_out.
  - Tail (layers 1-2) re-associates the SPADE affine so t=(h-mu) and
    u=(gamma+opg)*t run on DVE concurrently with the ACT istd chain; the
    beta conv biases are folded into the next conv's softplus bias
    host-side (W @ b_be), leaving one stt per half after istd.
  - Inputs are packed into few tensors and ordered so phase A's DMAs land
    first; late-needed weights load after the first collective.
  - Teardown control: exactly 3 tile pools, PSUM pre-allocated as 8 bank
    tiles rotated manually, tail scratch packed into few tiles — each
    pool.tile() call costs a serialized release barrier at kernel end.
  - Run-to-run exec variance (~±12us) comes from the runtime's
    first-collective 8-core barrier absorbing core-launch skew
    (observed 15-52us); it overlaps phase A but gates the first gather.

The host side packs per-core shards / weight transposes (layout only) and
reassembles the [2,1,32,32] output from cores 0 and 4.
"""

import numpy as np

from concourse import bass, tile, mybir
from concourse.bass_utils import run_bass_kernel_spmd

F32 = mybir.dt.float32
BF16 = mybir.dt.bfloat16
BF16_NP = mybir.dt.np(BF16)
FP8 = mybir.dt.float8e4
FP8_NP = mybir.dt.np(FP8)
PM_DR = mybir.MatmulPerfMode.DoubleRow
AOT = mybir.ActivationFunctionType
ALU = mybir.AluOpType

# Problem dims
B, CM, CS, HP, WP, HI, WI, HM, NSEG = 2, 1536, 384, 32, 32, 448, 448, 128, 64
G = 4              # cores per batch group
SEGC = NSEG // G   # segments per core = 16
COC = CM // G      # gamma/beta out-channel chunk per core = 384
NPIX = HP * WP     # 1024
EPS = 1e-12
LN0_N = float(CM * NPIX)
LN1_N = float(8 * NPIX)
LN2_N = float(16 * NPIX)
PAYW = CS + NPIX   # 1408 payload cols: avg | M
PAYW4 = PAYW + 4   # + LN0 stats (f32 pair as bf16 bits) in row 0's tail

_NC_CACHE = {}


def _resize_matrix(n_in, n_out):
    """Row matrix of jax.image.resize(..., 'bilinear') for downsampling
    (antialiased triangle kernel, normalized rows). Verified vs jax."""
    scale = n_out / n_in
    p = np.arange(n_out, dtype=np.float64)[:, None]
    i = np.arange(n_in, dtype=np.float64)[None, :]
    center = (p + 0.5) / scale - 0.5
    w = np.maximum(0.0, 1.0 - np.abs(i - center) * scale)
    w = w / w.sum(axis=1, keepdims=True)
    return w.astype(np.float32)


def _split_sync_waits(nc, max_waits=1):
    """walrus in this container encodes at most one sync-wait per
    instruction; hoist extras onto preceding same-engine NoOps."""
    n = 0
    for fn in nc.m.functions:
        for blk in fn.blocks:
            new_insts = []
            for inst in blk.instructions:
                si = getattr(inst, "sync_info", None)
                if si is not None and si.on_wait and len(si.on_wait) > max_waits:
                    waits = list(si.on_wait)
                    head, rest = waits[:-max_waits], waits[-max_waits:]
                    for i in range(0, len(head), max_waits):
                        new_insts.append(mybir.InstNoOp(
                            name=f"I-ws-{nc.next_id()}", engine=inst.engine,
                            ins=[], outs=[],
                            sync_info=mybir.SyncInfo(
                                on_wait=list(head[i:i + max_waits]), on_update=[]),
                        ))
                    si.on_wait = rest
                    n += 1
                new_insts.append(inst)
            blk.instructions = new_insts
    return n


def _conv_windows(pad_ap, rows, cols, dy, dx, row0=0):
    """AP view [P, rows, cols] of a padded [P, 34, 34] image at tap (dy,dx)."""
    return pad_ap[:, row0 + dy:row0 + dy + rows, dx:dx + cols]


def _build_nc():
    nc = bass.Bass()

    def inp(name, shape, dtype):
        return nc.declare_dram_parameter(name, list(shape), dtype, isOutput=False)

    # --- inputs (per-core packed shards; see _pack_inputs) ---
    segbf = inp("segbf", [112, 4, 448], BF16)
    rtct = inp("rtct", [112, 8, 32], BF16)       # resize mats: rt @0:4, ct @4:8
    rtct8 = inp("rtct8", [112, 2, 2, 32], FP8)   # R chunks in DR (c,c+1) pairs
    fsems = inp("fsems", [128, 8, 386], BF16)    # f_semT+ones | segsm @col 385
    xq = inp("xq", [128, 3, NPIX], BF16)
    # shared convs in fp8 DoubleRow: 6 pairs per cic; pair p<3 = taps
    # (p, p+3) i.e. (dy=0,dy=1) at dx=p; pair p>=3 = (tap 6+(p-3), zero)
    wsh0 = inp("wsh0", [128, 3, 6, 2, 128], FP8)
    wshm = inp("wshm", [128, 3, 6, 2, 128], FP8)
    # L0 gamma/beta weights, fp8, packed in DoubleRow tap pairs:
    # [ci, pair(5), plane(2), co_local]; pair 4 plane 1 is zeros
    wg = inp("wg", [128, 5, 2, COC], FP8)
    wbe = inp("wbe", [128, 5, 2, COC], FP8)
    wc0t = inp("wc0t", [128, 3, 8], BF16)
    wgbe1 = inp("wgbe1", [128, 9, 40], BF16)   # gamma @0:8, beta @32:40
    wgbe2 = inp("wgbe2", [128, 9, 48], BF16)   # gamma @0:16, beta @32:48
    # packed small tensors (see _pack_inputs for layouts)
    smallf = inp("smallf", [128, 41], F32)
    obc = inp("obc", [16, 384], F32)
    smallp = inp("smallp", [16, 7], F32)
    smallb = inp("smallb", [16, 17], BF16)

    out_t = nc.declare_dram_parameter("out", [1, NPIX], F32, isOutput=True)

    with tile.TileContext(nc) as tc:
        with (
            tc.tile_pool(name="work", bufs=1) as wpool,
            tc.tile_pool(name="dram", bufs=1, space="DRAM") as dpool,
            tc.tile_pool(name="ps", bufs=1, space="PSUM") as psB,
        ):
            # every pool costs ~2 serialized all-engine barrier groups in
            # the end-of-kernel teardown: keep exactly 3 pools and manage
            # buffer rotation manually via tags/slices
            cpool = wpool
            segpool = wpool
            wA = wpool

            # ---------- load constants / inputs into SBUF ----------
            def load(pool, ap, dtype=None, name=None):
                t = pool.tile(list(ap.shape), dtype or ap.dtype, tag=name)
                nc.sync.dma_start(out=t[:], in_=ap[:])
                return t

            seg_sb = load(wA, segbf, name="seg_sb")
            smallf_sb = load(cpool, smallf, name="smallf_sb")
            rtct_sb = load(wA, rtct, name="rtct_sb")
            rtct8_sb = load(wA, rtct8, name="rtct8_sb")
            fsems_sb = load(wA, fsems, name="fsems_sb")
            x_sb = load(cpool, xq, name="x_sb")
            obc_sb = load(cpool, obc, name="obc_sb")
            smallp_sb = load(cpool, smallp, name="smallp_sb")
            smallb_sb = load(cpool, smallb, name="smallb_sb")

            # views into the packed small tensors
            ones128f_sb = smallf_sb[:, 0:1]
            opg0_v = lambda m: smallf_sb[:, 1 + m:2 + m]
            bbe0a_v = lambda m: smallf_sb[:, 4 + m:5 + m]
            bsh0_v = smallf_sb[:, 7:8]
            bshm_v = smallf_sb[:, 8:9]
            segval_v = lambda s, n=128: smallf_sb[0:n, 9 + s:10 + s]
            negsegval_v = lambda s: smallf_sb[0:112, 25 + s:26 + s]
            obc0_v = obc_sb[0:G, 0:128]
            obc1_v = obc_sb[0:8, 128:256]
            obc2_v = obc_sb[0:16, 256:384]
            opg1_v, bbe1a_v = smallp_sb[0:8, 0:1], smallp_sb[0:8, 1:2]
            opg2_v, bbe2a_v = smallp_sb[0:16, 2:3], smallp_sb[0:16, 3:4]
            b0_v, b1_v, b2_v = (smallp_sb[0:8, 4:5], smallp_sb[0:16, 5:6],
                                smallp_sb[0:1, 6:7])
            wc1t_v = smallb_sb[0:8, 0:16]
            wc2t_v = smallb_sb[0:16, 16:17]

            # DRAM scratch
            crs_in = dpool.tile([SEGC, PAYW4], BF16)
            crs_out = dpool.tile([G, SEGC, PAYW4], BF16)
            crs2_in = dpool.tile([128, NPIX], BF16)
            crs2_out = dpool.tile([G, 128, NPIX], BF16)
            cc0_in = dpool.tile([8, NPIX], BF16)
            cc0_out = dpool.tile([8, NPIX], BF16)


            # Pre-allocate the 8 PSUM banks ONCE and rotate manually: every
            # pool.tile() call emits a TileRelease that drains serially in
            # the end-of-kernel teardown (~165ns each); phase A's tiles are
            # slices of the same 8 buffers. Dep tracking per buffer is
            # identical to pool rotation.
            bigA = [psB.tile([128, 512], F32, tag=f"bigA{i}", name=f"bigA{i}")
                    for i in range(4)]
            bigB = [psB.tile([128, 512], F32, tag=f"bigB{i}", name=f"bigB{i}")
                    for i in range(2)]
            accC = [psB.tile([128, 512], F32, tag=f"accC{i}", name=f"accC{i}")
                    for i in range(2)]
            _rot = {"A": 0, "B": 0, "C": 0}

            def take(which):
                lst = {"A": bigA, "B": bigB, "C": accC}[which]
                t = lst[_rot[which] % len(lst)]
                _rot[which] += 1
                return t

            # ---------- Phase A3: M matrices for my 16 segments ----------
            mall_ps = accC[0][0:32, :]
            for s in range(SEGC):
                oh = segpool.tile([112, 4, 448], FP8, tag="oh", bufs=3)
                # split the one-hot compares across DVE and the otherwise
                # idle GpSimd engine (phase A is DVE-bound after the DR
                # matmul cut)
                eng = nc.gpsimd if s % 3 == 2 else nc.vector
                eng.tensor_scalar(oh[:], seg_sb[:],
                                  segval_v(s, 112), None,
                                  ALU.is_equal)
                # A^T[j,p] = sum_i oh[i,j] R[p,i]; the one-hot (0/1, exact
                # in fp8) is the DR lhsT with i-chunk pairs as the K
                # extension — halves the LDWEIGHTS-bound matmul count
                at_ps = bigA[s % 3][0:112, 0:128].rearrange(
                    "p (c q) -> p c q", c=4)
                for jb in range(4):
                    for pr in range(2):
                        nc.tensor.matmul(
                            at_ps[:, jb, :],
                            lhsT=oh[:, 2 * pr:2 * pr + 2,
                                    jb * 112:(jb + 1) * 112],
                            rhs=rtct8_sb[:, pr, :, :],
                            start=(pr == 0), stop=(pr == 1),
                            perf_mode=PM_DR)
                at_sb = segpool.tile([112, 4, 32], BF16, tag="atsb", bufs=3)
                nc.scalar.activation(at_sb[:], at_ps[:], AOT.Copy)
                for c in range(4):
                    nc.tensor.matmul(mall_ps[:, s * 32:(s + 1) * 32],
                                     lhsT=at_sb[:, c, :], rhs=rtct_sb[:, 4 + c, :],
                                     start=(c == 0), stop=(c == 3))

            # M [32(p), 16(s), 32(q)] -> payload rows [16(s), p*32+q]
            mall_bf = wpool.tile([32, SEGC, 32], BF16, tag="mall_bf")
            nc.vector.tensor_copy(
                mall_bf[:], mall_ps[:].rearrange("p (s q) -> p s q", s=SEGC))
            nc.sync.dma_start(
                out=crs_in[0:SEGC, CS:PAYW].rearrange(
                    "s (p q) -> p s q", p=32),
                in_=mall_bf[:])

            # ---------- Phase A1: LayerNorm0 partial stats from x chunk ----------
            xsum = wpool.tile([128, 1], F32, tag="xsum")
            nc.vector.tensor_reduce(xsum[:], x_sb[:], mybir.AxisListType.XY, ALU.add)
            xsq_scratch = wpool.tile([128, 3, NPIX], BF16, tag="sq_scratch")
            xsumsq = wpool.tile([128, 1], F32, tag="xsumsq")
            nc.scalar.activation(xsq_scratch[:], x_sb[:], AOT.Square,
                                 accum_out=xsumsq[:])
            stats2 = wpool.tile([128, 2], F32, tag="stats2")
            nc.vector.tensor_copy(stats2[:, 0:1], xsum[:])
            nc.vector.tensor_copy(stats2[:, 1:2], xsumsq[:])

            # cross-partition reduce of LN0 partial stats via ones-matmul
            stats1_ps = bigB[1][0:1, 0:2]
            nc.tensor.matmul(stats1_ps[:], lhsT=ones128f_sb, rhs=stats2[:],
                             start=True, stop=True)
            stats1_sb = wpool.tile([1, 2], F32, tag="stats1_sb")
            nc.scalar.activation(stats1_sb[:], stats1_ps[:], AOT.Copy)
            # ride the AllGather payload as raw bits (bf16 view of f32)
            nc.sync.dma_start(out=crs_in[0:1, PAYW:PAYW4],
                              in_=stats1_sb[:].bitcast(BF16))

            # ---------- Phase A2: segment averages for my 16 segments ----------
            ohsm = wpool.tile([128, SEGC, 8], BF16, tag="ohsm")
            for s in range(SEGC):
                nc.vector.tensor_scalar(ohsm[:, s, :], fsems_sb[:, :, 385],
                                        segval_v(s), None, ALU.is_equal)
            sums_ps = bigB[0][0:SEGC, 0:385]
            for c in range(8):
                nc.tensor.matmul(sums_ps[:], lhsT=ohsm[:, :, c],
                                 rhs=fsems_sb[:, c, 0:385],
                                 start=(c == 0), stop=(c == 7))
            sums_sb = wpool.tile([SEGC, 385], F32, tag="sums_sb")
            nc.scalar.activation(sums_sb[:], sums_ps[:], AOT.Copy)
            cnt_safe = wpool.tile([SEGC, 1], F32, tag="cnt_safe")
            nc.vector.tensor_scalar(cnt_safe[:], sums_sb[:, 384:385], 1.0, None,
                                    ALU.max)
            rec = wpool.tile([SEGC, 1], F32, tag="rec")
            nc.vector.reciprocal(rec[:], cnt_safe[:])
            mask = wpool.tile([SEGC, 1], F32, tag="mask")
            nc.vector.tensor_scalar(mask[:], sums_sb[:, 384:385], 0.5, None,
                                    ALU.is_gt)
            recm = wpool.tile([SEGC, 1], F32, tag="recm")
            nc.vector.tensor_mul(recm[:], rec[:], mask[:])
            avg_bf = wpool.tile([SEGC, CS], BF16, tag="avg_bf")
            nc.vector.tensor_scalar(avg_bf[:], sums_sb[:, 0:384], recm[:], None,
                                    ALU.mult)
            nc.sync.dma_start(out=crs_in[0:SEGC, 0:CS], in_=avg_bf[:])

            # ---------- C1: ONE small AllGather of (avg | M | stats) ----------
            nc.gpsimd.collective_compute(
                "AllGather", ALU.bypass,
                replica_groups=[[0, 1, 2, 3], [4, 5, 6, 7]],
                ins=[crs_in[:]], outs=[crs_out[:]],
            )

            wB = wpool

            # weights not needed until ~40us after the gather: load them
            # here so the pre-barrier input DMA stream stays short
            wshm_sb = load(wB, wshm, name="wshm_sb")
            wsh0_sb = load(wB, wsh0, name="wsh0_sb")
            wg_sb = load(wB, wg, name="wg_sb")
            wbe_sb = load(wB, wbe, name="wbe_sb")
            wc0t_sb = load(cpool, wc0t, name="wc0t_sb")
            wgbe1_sb = load(wB, wgbe1, name="wgbe1_sb")
            wgbe2_sb = load(wB, wgbe2, name="wgbe2_sb")

            # small PE filler after phase A keeps the HAM clock gate open a
            # bit longer into the gather window
            for i in range(8):
                fl = take("B")
                nc.tensor.matmul(fl[:], lhsT=x_sb[:, 0, 0:128],
                                 rhs=x_sb[:, 0, 0:512],
                                 start=True, stop=True)

            # ---------- Phase B0: unpack gather + LN0 scalars ----------
            avg_all = wB.tile([NSEG, CS], BF16, tag="avg_all")
            m_all = wB.tile([NSEG, NPIX], BF16, tag="m_all")
            nc.sync.dma_start(
                out=avg_all[:],
                in_=crs_out[:, :, 0:CS].rearrange("g s c -> (g s) c"))
            nc.sync.dma_start(
                out=m_all[:],
                in_=crs_out[:, :, CS:PAYW].rearrange("g s c -> (g s) c"))
            stats4 = wB.tile([G, 4], BF16, tag="stats4")
            nc.sync.dma_start(out=stats4[:], in_=crs_out[:, 0, PAYW:PAYW4])

            # reduce the 4 partial stats, divide by N (folded into the ones
            # block) AND broadcast to 128 partitions in one matmul
            st0_ps = take("C")[:, 0:2]
            nc.tensor.matmul(st0_ps[:], lhsT=obc0_v,
                             rhs=stats4[:].bitcast(F32), start=True, stop=True)
            # LN0 scalars live long (used across all m chunks): copy to SBUF
            st0_bc = wB.tile([128, 2], F32, tag="st0_bc")
            nc.scalar.activation(st0_bc[:], st0_ps[:], AOT.Copy)

            def ln_from_ms(ms_ap, nparts, scr):
                """ms_ap [nparts,2]=(mean, meansq); scr [nparts,4] F32
                scratch (musq|vare|lnv|istd) -> (mu AP, istd AP)."""
                musq, vare = scr[:, 0:1], scr[:, 1:2]
                lnv, istd = scr[:, 2:3], scr[:, 3:4]
                nc.scalar.activation(musq, ms_ap[:, 0:1], AOT.Square)
                nc.vector.scalar_tensor_tensor(vare, ms_ap[:, 1:2], EPS,
                                               musq, ALU.add, ALU.subtract)
                nc.scalar.activation(lnv, vare, AOT.Ln)
                nc.scalar.activation(istd, lnv, AOT.Exp, scale=-0.5)
                return ms_ap[:, 0:1], istd

            ln0scr = wB.tile([128, 4], F32, tag="ln0scr")
            mu0_bc, istd0_bc = ln_from_ms(st0_bc, 128, ln0scr[:])

            def pad_borders(t33):
                """zero only the 1-px border ring (interior is overwritten)."""
                nc.vector.memset(t33[:, 0:1, :], 0.0)
                nc.vector.memset(t33[:, 33:34, :], 0.0)
                nc.vector.memset(t33[:, 1:33, 0:1], 0.0)
                nc.vector.memset(t33[:, 1:33, 33:34], 0.0)

            # ---------- Phase B1: full sem_rs, padded, in SBUF (bf16) ----------
            semrs_pad = []
            for k in range(3):
                sp = wpool.tile([128, 34, 34], BF16, tag=f"semrs_pad{k}")
                pad_borders(sp)
                for h in range(2):
                    ps = take("A")
                    nc.tensor.matmul(ps[:],
                                     lhsT=avg_all[:, k * 128:(k + 1) * 128],
                                     rhs=m_all[:, h * 512:(h + 1) * 512],
                                     start=True, stop=True)
                    nc.scalar.activation(
                        sp[:, 1 + h * 16:17 + h * 16, 1:33],
                        ps[:].rearrange("c (r w) -> c r w", r=16), AOT.Copy)
                semrs_pad.append(sp)

            # ---------- shared 3x3 convs ----------
            # L0 is computed locally by every rank; ranks 0/1 compute the
            # full L1, ranks 2/3 the full L2, then an AllGather (hidden
            # under the L0 gamma/beta convs) distributes them.
            hsh_pad = [wpool.tile([128, 34, 34], BF16, tag=f"hsh_pad{L}",
                                  name=f"hsh_pad{L}")
                       for L in range(3)]

            def dr_win(sp, h, pr):
                """[128,2,16,32] view: plane 1 is plane 0 shifted one image
                row down (stride 48B). For pr>=3 plane 1 reads in-bounds
                garbage rows whose weights are zero."""
                dy0 = 0 if pr < 3 else 2
                dx = pr % 3
                ap = sp[:, h * 16 + dy0:h * 16 + dy0 + 16,
                        dx:dx + 32].copy()
                ap.ap.insert(1, (48, 2))
                return ap

            def sh_conv_half(ps, w_sb, h):
                for cic in range(3):
                    for pr in range(6):
                        nc.tensor.matmul(
                            ps[:].rearrange("c (r w) -> c r w", r=16),
                            lhsT=w_sb[:, cic, pr, :, :],
                            rhs=dr_win(semrs_pad[cic], h, pr),
                            start=(cic == 0 and pr == 0),
                            stop=(cic == 2 and pr == 5),
                            perf_mode=PM_DR)

            # my rank-pair's layer first, so the AllGather can start early
            hshm_flat = wpool.tile([128, NPIX], BF16, tag="hshm_flat")
            for h in range(2):
                ps = take("A")
                sh_conv_half(ps, wshm_sb, h)
                nc.scalar.activation(hshm_flat[:, h * 512:(h + 1) * 512],
                                     ps[:], AOT.Relu, bias=bshm_v)
            nc.sync.dma_start(out=crs2_in[:], in_=hshm_flat[:])
            nc.gpsimd.collective_compute(
                "AllGather", ALU.bypass,
                replica_groups=[[0, 1, 2, 3], [4, 5, 6, 7]],
                ins=[crs2_in[:]], outs=[crs2_out[:]],
            )

            # L0 locally
            pad_borders(hsh_pad[0])
            for h in range(2):
                ps = take("A")
                sh_conv_half(ps, wsh0_sb, h)
                nc.scalar.activation(
                    hsh_pad[0][:, 1 + h * 16:17 + h * 16, 1:33],
                    ps[:].rearrange("c (r w) -> c r w", r=16), AOT.Relu,
                    bias=bsh0_v)

            # ---------- Phase B5/B6: layer-0 gamma/beta convs + SPADE + c0 ----
            c0_ps = [take("C")[0:8, :] for h in range(2)]
            # fp8 interleaved window images (DoubleRow rhs): per half,
            # [pair(5), plane(2), 512]; pair p covers taps (2p, 2p+1),
            # pair 4 plane 1 is zeros
            im8 = [wpool.tile([128, 5, 2, 512], FP8, tag=f"im8_{h}",
                              name=f"im8_{h}") for h in range(2)]
            for h in range(2):
                for t in range(9):
                    dy, dx = t // 3, t % 3
                    nc.vector.tensor_copy(
                        im8[h][:, t // 2, t % 2, :].rearrange(
                            "c (r w) -> c r w", r=16),
                        _conv_windows(hsh_pad[0][:], 16, 32, dy, dx,
                                      row0=h * 16))
                nc.vector.memset(im8[h][:, 4, 1, :], 0.0)
            for m in range(3):
                xn_m = wpool.tile([128, NPIX], BF16, tag="xn", bufs=2)
                nc.vector.tensor_scalar(xn_m[:], x_sb[:, m, :], mu0_bc[:],
                                        istd0_bc[:], ALU.subtract, ALU.mult)
                for h in range(2):
                    gb_ps = {}
                    for name, w_sb in (("g", wg_sb), ("be", wbe_sb)):
                        ps = take("A") if name == "g" else take("B")
                        for pr in range(5):
                            nc.tensor.matmul(
                                ps[:],
                                lhsT=w_sb[:, pr, :, m * 128:(m + 1) * 128],
                                rhs=im8[h][:, pr, :, :],
                                start=(pr == 0), stop=(pr == 4),
                                perf_mode=PM_DR)
                        gb_ps[name] = ps
                    # spade: out = xn*(1+gamma+bg) + (beta+bbe)
                    hs = slice(h * 512, (h + 1) * 512)
                    u = wpool.tile([128, 512], BF16, tag="spade_u", bufs=2)
                    nc.vector.scalar_tensor_tensor(u[:], gb_ps["g"][:],
                                                   opg0_v(m),
                                                   xn_m[:, hs],
                                                   ALU.add, ALU.mult)
                    sp0 = wpool.tile([128, 512], BF16, tag="spade_o", bufs=2)
                    nc.vector.scalar_tensor_tensor(sp0[:], gb_ps["be"][:],
                                                   bbe0a_v(m), u[:],
                                                   ALU.add, ALU.add)
                    nc.tensor.matmul(c0_ps[h][:], lhsT=wc0t_sb[:, m, :],
                                     rhs=sp0[:], start=(m == 0), stop=(m == 2))

            c0p_sb = wpool.tile([8, NPIX], BF16, tag="f32buf")
            for h in range(2):
                nc.scalar.activation(c0p_sb[:, h * 512:(h + 1) * 512],
                                     c0_ps[h][:], AOT.Copy)
            nc.sync.dma_start(out=cc0_in[:], in_=c0p_sb[:])
            nc.gpsimd.collective_compute(
                "AllReduce", ALU.add,
                replica_groups=[[0, 1, 2, 3], [4, 5, 6, 7]],
                ins=[cc0_in[:]], outs=[cc0_out[:]],
            )

            # emitted after the collective trigger: unpack gathered sh layers
            # and prefire the L1/L2 conv matmuls so they overlap C2 on the PE
            hshg = [None, None]
            for L in (1, 2):
                hg = wpool.tile([128, NPIX], BF16, tag=f"hshg{L}",
                                name=f"hshg{L}")
                nc.sync.dma_start(out=hg[:], in_=crs2_out[0 if L == 1 else 2])
                hp = hsh_pad[L]
                pad_borders(hp)
                nc.vector.tensor_copy(
                    hp[:, 1:33, 1:33],
                    hg[:].rearrange("c (r w) -> c r w", r=32))
                hshg[L - 1] = hg

            def spade_conv_ps(nco, wgbe_sb, pad_img):
                pss = []
                for h in range(2):
                    p = take("A")[0:32 + nco, :]
                    for t in range(9):
                        dy, dx = t // 3, t % 3
                        nc.tensor.matmul(
                            p[:].rearrange("c (r w) -> c r w", r=16),
                            lhsT=wgbe_sb[:, t, :],
                            rhs=_conv_windows(pad_img[:], 16, 32, dy, dx,
                                              row0=h * 16),
                            start=(t == 0), stop=(t == 8))
                    pss.append(p)
                return pss

            l1_ps = spade_conv_ps(8, wgbe1_sb, hsh_pad[1])
            l2_ps = spade_conv_ps(16, wgbe2_sb, hsh_pad[2])


            # ---------- Phase B7: h1 + LN1 ----------
            # tail scratch is packed into few tiles (releases are per tile)
            spx_e = wpool.tile([16, NPIX], F32, tag="spx_e")
            lnpk = wpool.tile([16, 8], F32, tag="lnpk")
            st2pk = wpool.tile([16, 4], F32, tag="st2pk")
            # h1/h2/out overlap at partition 0: each is fully dead
            # (last read by t/Square) before the next one is written
            hpk = wpool.tile([16, NPIX], F32, tag="hpk")
            tpk = wpool.tile([16, NPIX], BF16, tag="tpk")
            upk = wpool.tile([16, NPIX], BF16, tag="upk")
            sppk = wpool.tile([16, NPIX], BF16, tag="sppk")

            def softplus_to(dst, src_aps, bias_ap, nparts, tag,
                            accum_out=None):
                """dst = ln(1 + exp(src + bias)); inputs here are small, so
                exp cannot overflow and both ACTs share one table set.
                src_aps: list of (ap, col0) slices of any width (PSUM/SBUF).
                accum_out, if given, receives sum(dst) for free."""
                for ap, col0 in src_aps:
                    w = ap.shape[-1]
                    nc.scalar.activation(spx_e[0:nparts, col0:col0 + w],
                                         ap, AOT.Exp, bias=bias_ap)
                nc.scalar.activation(dst[:], spx_e[0:nparts, :], AOT.Ln,
                                     bias=1.0, accum_out=accum_out)

            c0_sb = wpool.tile([8, NPIX], BF16, tag="f32buf2")
            nc.sync.dma_start(out=c0_sb[:], in_=cc0_out[:])
            st2_1 = st2pk[0:8, 0:2]
            h1_f32 = hpk[0:8, :]
            softplus_to(h1_f32, [(c0_sb[:, 0:NPIX], 0)],
                        b0_v, 8, "sp1", accum_out=st2_1[:, 0:1])

            def ln_small(h_f32, nparts, obc_sb, st2, scr):
                # Square's dst is never read (only accum_out matters):
                # dump it into spx_e, whose last reader has already run
                nc.scalar.activation(spx_e[0:nparts, :], h_f32[:], AOT.Square,
                                     accum_out=st2[:, 1:2])
                st_ps = take("C")[:, 0:2]
                nc.tensor.matmul(st_ps[:], lhsT=obc_sb[:],
                                 rhs=st2[:], start=True, stop=True)
                return ln_from_ms(st_ps[0:nparts, :], nparts, scr)

            # ---------- Phase B8: layers 1 and 2 (replicated) ----------
            def spade_small(h_f32, mu_ap, istd, nco, pss, opg_ap, tag):
                """spade with the LN affine re-associated off the critical
                path: t=(h-mu) and u=(gamma+opg)*t run on DVE while the ACT
                istd chain computes; only sp = u*istd + beta remains after
                istd. The beta conv BIAS is folded into the next stage's
                softplus bias host-side (W @ b_be)."""
                t_ = tpk[0:nco, :]
                nc.vector.tensor_scalar(t_[:], h_f32[:], mu_ap, None,
                                        ALU.subtract)
                sp_ = sppk[0:nco, :]
                for h in range(2):
                    hs = slice(h * 512, (h + 1) * 512)
                    nc.vector.scalar_tensor_tensor(upk[0:nco, hs],
                                                   pss[h][0:nco, :],
                                                   opg_ap, t_[:, hs],
                                                   ALU.add, ALU.mult)
                for h in range(2):
                    hs = slice(h * 512, (h + 1) * 512)
                    nc.vector.scalar_tensor_tensor(sp_[:, hs],
                                                   upk[0:nco, hs],
                                                   istd[:],
                                                   pss[h][32:32 + nco, :],
                                                   ALU.mult, ALU.add)
                return sp_

            mu1_bc, istd1_bc = ln_small(h1_f32, 8, obc1_v, st2_1,
                                        lnpk[0:8, 0:4])
            sp1 = spade_small(h1_f32, mu1_bc, istd1_bc, 8, l1_ps, opg1_v, "l1")
            c1_pss = []
            for h in range(2):
                c1_ps = take("C")[0:16, :]
                nc.tensor.matmul(c1_ps[:], lhsT=wc1t_v,
                                 rhs=sp1[:, h * 512:(h + 1) * 512],
                                 start=True, stop=True)
                c1_pss.append((c1_ps[:], h * 512))
            st2_2 = st2pk[0:16, 2:4]
            h2_f32 = hpk[0:16, :]
            softplus_to(h2_f32, c1_pss, b1_v, 16, "sp2",
                        accum_out=st2_2[:, 0:1])

            mu2_bc, istd2_bc = ln_small(h2_f32, 16, obc2_v, st2_2,
                                        lnpk[0:16, 4:8])
            sp2 = spade_small(h2_f32, mu2_bc, istd2_bc, 16, l2_ps, opg2_v,
                              "l2")
            c2_pss = []
            for h in range(2):
                c2_ps = take("C")[0:1, :]
                nc.tensor.matmul(c2_ps[:], lhsT=wc2t_v,
                                 rhs=sp2[:, h * 512:(h + 1) * 512],
                                 start=True, stop=True)
                c2_pss.append((c2_ps[:], h * 512))
            out_f32 = hpk[0:1, :]
            softplus_to(out_f32, c2_pss, b2_v, 1, "sp3")
            nc.sync.dma_start(out=out_t[:], in_=out_f32[:])

    nc.finalize()
    _split_sync_waits(nc)
    return nc


def _pack_inputs(inputs):
    f32 = np.float32
    R = _resize_matrix(HI, HP)      # [32, 448]
    C = _resize_matrix(WI, WP)      # [32, 448]
    rtct = np.zeros((112, 8, 32), f32)
    for c in range(4):
        rtct[:, c, :] = R[:, c * 112:(c + 1) * 112].T
        rtct[:, 4 + c, :] = C[:, c * 112:(c + 1) * 112].T
    rtct8 = np.zeros((112, 2, 2, 32), f32)
    for c in range(4):
        rtct8[:, c // 2, c % 2, :] = R[:, c * 112:(c + 1) * 112].T
    rtct8 = rtct8.astype(FP8_NP)
    rtct = rtct.astype(BF16_NP)

    segmap = inputs["segmap"]            # [2, 448, 448] int32
    f_sem = inputs["f_semantic"]         # [2, 384, 32, 32]
    x_main = inputs["x_main"]            # [2, 1536, 32, 32]
    rows = (np.arange(HP) * HI) // HP

    def tap_t(w):  # [co, ci, 3, 3] -> [ci, 9, co]
        return np.ascontiguousarray(w.transpose(1, 2, 3, 0).reshape(
            w.shape[1], 9, w.shape[0]))

    # shared conv layers, fp8 DR pairs: [ci, cic, pair, plane, co]
    def sh_pack(L):
        wt = tap_t(inputs[f"w_sh{L}"]).reshape(3, 128, 9, HM)  # cic,ci,t,co
        out = np.zeros((128, 3, 6, 2, HM), np.float32)
        for pr in range(3):
            out[:, :, pr, 0, :] = wt[:, :, pr, :].transpose(1, 0, 2)
            out[:, :, pr, 1, :] = wt[:, :, pr + 3, :].transpose(1, 0, 2)
            out[:, :, 3 + pr, 0, :] = wt[:, :, 6 + pr, :].transpose(1, 0, 2)
        return out.astype(FP8_NP)
    wsh_pk = [sh_pack(L) for L in range(3)]
    bsh_pk = [inputs[f"b_sh{L}"].reshape(128,) for L in range(3)]

    def gbe_pack(wg_, wbe_, nco):   # [128, 9, 32+nco], beta at col 32
        out = np.zeros((128, 9, 32 + nco), f32)
        out[:, :, 0:nco] = tap_t(wg_)
        out[:, :, 32:32 + nco] = tap_t(wbe_)
        return out.astype(BF16_NP)

    wgbe1 = gbe_pack(inputs["w_g1"], inputs["w_be1"], 8)
    wgbe2 = gbe_pack(inputs["w_g2"], inputs["w_be2"], 16)

    # smallp [16, 7]: opg1|bbe1a|opg2|bbe2a|b0|b1|b2
    smallp = np.zeros((16, 7), f32)
    smallp[0:8, 0] = 1.0 + inputs["b_g1"]
    smallp[0:8, 1] = inputs["b_be1"]
    smallp[0:16, 2] = 1.0 + inputs["b_g2"]
    smallp[0:16, 3] = inputs["b_be2"]
    smallp[0:8, 4] = inputs["bias0"]
    # beta conv biases of layers 1/2 are dropped from the spade output and
    # folded into the next pointwise conv's bias: W @ b_be is a constant
    smallp[0:16, 5] = (inputs["bias1"]
                       + inputs["w_c1"][:, :, 0, 0] @ inputs["b_be1"])
    smallp[0:1, 6] = (inputs["bias2"]
                      + inputs["w_c2"][:, :, 0, 0] @ inputs["b_be2"])

    # smallb [16, 17]: wc1t | wc2t
    smallb = np.zeros((16, 17), f32)
    smallb[0:8, 0:16] = inputs["w_c1"][:, :, 0, 0].T
    smallb[0:16, 16] = inputs["w_c2"][0, :, 0, 0]
    smallb = smallb.astype(BF16_NP)

    # obc [16, 384]: ones/N blocks for the LN reduce+broadcast matmuls
    obc = np.zeros((16, 384), f32)
    obc[0:4, 0:128] = 1.0 / LN0_N
    obc[0:8, 128:256] = 1.0 / LN1_N
    obc[0:16, 256:384] = 1.0 / LN2_N

    maps = []
    for cid in range(8):
        b, g = cid // G, cid % G
        d = {}
        seg = segmap[b].astype(f32)
        d["segbf"] = seg.reshape(4, 112, WI).transpose(1, 0, 2).astype(BF16_NP)
        d["rtct"] = rtct
        d["rtct8"] = rtct8
        seg_small = seg[rows[:, None], rows[None, :]].reshape(-1)   # [1024]
        segsm = seg_small.reshape(8, 128).T                          # [128, 8]
        fT = f_sem[b].reshape(CS, NPIX).T                           # [1024, 384]
        fTe = np.concatenate([fT, np.ones((NPIX, 1), f32)], 1)      # [1024, 385]
        fsems = np.zeros((128, 8, 386), f32)
        fsems[:, :, 0:385] = fTe.reshape(8, 128, 385).transpose(1, 0, 2)
        fsems[:, :, 385] = segsm
        d["fsems"] = fsems.astype(BF16_NP)
        xc = x_main[b, g * COC:(g + 1) * COC].reshape(COC, NPIX)
        d["xq"] = np.ascontiguousarray(
            xc.reshape(3, 128, NPIX).transpose(1, 0, 2)).astype(BF16_NP)
        d["wsh0"] = wsh_pk[0]
        mine = 1 if g < 2 else 2
        d["wshm"] = wsh_pk[mine]
        cosl = slice(g * COC, (g + 1) * COC)
        def dr_pack(w):  # [128,9,384] -> [128, 5, 2, 384] fp8, tap pairs
            wt = tap_t(w)
            out = np.zeros((128, 5, 2, COC), np.float32)
            for t in range(9):
                out[:, t // 2, t % 2, :] = wt[:, t, :]
            return out.astype(FP8_NP)

        d["wg"] = dr_pack(inputs["w_g0"][cosl])
        d["wbe"] = dr_pack(inputs["w_be0"][cosl])
        wc0 = inputs["w_c0"][:, :, 0, 0]                           # [8, 1536]
        d["wc0t"] = np.ascontiguousarray(
            wc0[:, cosl].T.reshape(3, 128, 8).transpose(1, 0, 2)).astype(BF16_NP)
        d["wgbe1"] = wgbe1
        d["wgbe2"] = wgbe2
        # smallf [128, 41]: ones | opg0(3) | bbe0a(3) | bsh0 | bshm |
        # segval(16) | -segval(16)
        smallf = np.zeros((128, 41), f32)
        smallf[:, 0] = 1.0
        smallf[:, 1:4] = (1.0 + inputs["b_g0"][cosl]).reshape(3, 128).T
        smallf[:, 4:7] = inputs["b_be0"][cosl].reshape(3, 128).T
        smallf[:, 7] = bsh_pk[0]
        smallf[:, 8] = bsh_pk[mine]
        segv = (np.arange(SEGC, dtype=f32) + SEGC * g)[None, :]
        smallf[:, 9:25] = segv
        smallf[:, 25:41] = -segv
        d["smallf"] = smallf
        d["obc"] = obc
        d["smallp"] = smallp
        d["smallb"] = smallb
        maps.append(d)
    return maps


def kernel(**inputs):
    if "nc" not in _NC_CACHE:
        _NC_CACHE["nc"] = _build_nc()
    nc = _NC_CACHE["nc"]
    in_maps = _pack_inputs(inputs)
    res = run_bass_kernel_spmd(nc, in_maps, list(range(8)))
    out = np.zeros((B, 1, HP, WP), np.float32)
    out[0, 0] = res.results[0]["out"].reshape(HP, WP)
    out[1, 0] = res.results[4]["out"].reshape(HP, WP)
    return out


if __name__ == "__main__":
    nc = _build_nc()
    print("built OK; instructions:",
          sum(len(b.instructions) for f in nc.m.functions for b in f.blocks))



# revision 52
# speedup vs baseline: 1.8386x; 1.8386x over previous
"""Trainium2 Bass kernel for nn_DinoGazeSpade (segment_reduce + SPADE stack).

Strategy (8 NeuronCores, SPMD single program):
  - Two groups of 4 cores; group = batch index b (0..1), rank g = core % 4.
  - Painted-map + bilinear resize is reformulated as segment matrices:
        sem_rs[c,p,q] = sum_s avg[s,c] * M[s,p,q],
        M[s] = R @ onehot_s @ C^T   (R, C: 32x448 separable resize matrices)
    Each core builds avg + M for its 16 segments only, then ONE small
    in-group AllGather ships (avg ‖ M ‖ LN0 stats) [17,1408] bf16 (~48KB);
    every core then computes the FULL sem_rs with a cheap k=64 matmul set.
  - Shared 3x3 convs: every rank computes L0 locally; rank pairs (0,1)/(2,3)
    compute the full L1/L2 and an AllGather (hidden under the L0 gamma/beta
    convs) distributes them.
  - gamma/beta convs of layer 0 are split by output channel (384 per core),
    run in fp8 DoubleRow mode (2 taps per matmul stream); the pointwise c0
    conv partials are AllReduced in bf16 (C2), overlapped by the prefired
    L1/L2 conv matmuls.  Layers 1-2 are tiny and replicated.
  - conv3x3 = 9 shifted matmuls over a zero-padded [C,34,34] SBUF image.
  - LayerNorm scalars use a ones/N-matmul reduce+broadcast (no DRAM
    roundtrip); softplus = ln(1+exp(t)) in two ACT ops sharing one table
    set, with the LN sum folded into the Ln via accum_out.
  - Tail (layers 1-2) re-associates the SPADE affine so t=(h-mu) and
    u=(gamma+opg)*t run on DVE concurrently with the ACT istd chain; the
    beta conv biases are folded into the next conv's softplus bias
    host-side (W @ b_be), leaving one stt per half after istd.
  - Inputs are packed into few tensors and ordered so phase A's DMAs land
    first; late-needed weights load after the first collective.
  - Teardown control: exactly 3 tile pools, PSUM pre-allocated as 8 bank
    tiles rotated manually, tail scratch packed into few tiles — each
    pool.tile() call costs a serialized release barrier at kernel end.
  - Run-to-run exec variance (~±12us) comes from the runtime's
    first-collective 8-core barrier absorbing core-launch skew
    (observed 15-52us); it overlaps phase A but gates the first gather.

The host side packs per-core shards / weight transposes (layout only) and
reassembles the [2,1,32,32] output from cores 0 and 4.
"""

import numpy as np

from concourse import bass, tile, mybir
from concourse.bass_utils import run_bass_kernel_spmd

F32 = mybir.dt.float32
BF16 = mybir.dt.bfloat16
BF16_NP = mybir.dt.np(BF16)
FP8 = mybir.dt.float8e4
FP8_NP = mybir.dt.np(FP8)
PM_DR = mybir.MatmulPerfMode.DoubleRow
AOT = mybir.ActivationFunctionType
ALU = mybir.AluOpType

# Problem dims
B, CM, CS, HP, WP, HI, WI, HM, NSEG = 2, 1536, 384, 32, 32, 448, 448, 128, 64
G = 4              # cores per batch group
SEGC = NSEG // G   # segments per core = 16
COC = CM // G      # gamma/beta out-channel chunk per core = 384
NPIX = HP * WP     # 1024
EPS = 1e-12
LN0_N = float(CM * NPIX)
LN1_N = float(8 * NPIX)
LN2_N = float(16 * NPIX)
PAYW = CS + NPIX   # 1408 payload cols: avg | M
PAYW4 = PAYW + 4   # + LN0 stats (f32 pair as bf16 bits) in row 0's tail

_NC_CACHE = {}


def _resize_matrix(n_in, n_out):
    """Row matrix of jax.image.resize(..., 'bilinear') for downsampling
    (antialiased triangle kernel, normalized rows). Verified vs jax."""
    scale = n_out / n_in
    p = np.arange(n_out, dtype=np.float64)[:, None]
    i = np.arange(n_in, dtype=np.float64)[None, :]
    center = (p + 0.5) / scale - 0.5
    w = np.maximum(0.0, 1.0 - np.abs(i - center) * scale)
    w = w / w.sum(axis=1, keepdims=True)
    return w.astype(np.float32)


def _split_sync_waits(nc, max_waits=1):
    """walrus in this container encodes at most one sync-wait per
    instruction; hoist extras onto preceding same-engine NoOps."""
    n = 0
    for fn in nc.m.functions:
        for blk in fn.blocks:
            new_insts = []
            for inst in blk.instructions:
                si = getattr(inst, "sync_info", None)
                if si is not None and si.on_wait and len(si.on_wait) > max_waits:
                    waits = list(si.on_wait)
                    head, rest = waits[:-max_waits], waits[-max_waits:]
                    for i in range(0, len(head), max_waits):
                        new_insts.append(mybir.InstNoOp(
                            name=f"I-ws-{nc.next_id()}", engine=inst.engine,
                            ins=[], outs=[],
                            sync_info=mybir.SyncInfo(
                                on_wait=list(head[i:i + max_waits]), on_update=[]),
                        ))
                    si.on_wait = rest
                    n += 1
                new_insts.append(inst)
            blk.instructions = new_insts
    return n


def _conv_windows(pad_ap, rows, cols, dy, dx, row0=0):
    """AP view [P, rows, cols] of a padded [P, 34, 34] image at tap (dy,dx)."""
    return pad_ap[:, row0 + dy:row0 + dy + rows, dx:dx + cols]


def _build_nc():
    nc = bass.Bass()

    def inp(name, shape, dtype):
        return nc.declare_dram_parameter(name, list(shape), dtype, isOutput=False)

    # --- inputs (per-core packed shards; see _pack_inputs) ---
    segbf = inp("segbf", [112, 4, 448], BF16)
    rtct = inp("rtct", [112, 8, 32], BF16)       # resize mats: rt @0:4, ct @4:8
    rtct8 = inp("rtct8", [112, 2, 2, 32], FP8)   # R chunks in DR (c,c+1) pairs
    fsems = inp("fsems", [128, 8, 386], BF16)    # f_semT+ones | segsm @col 385
    xq = inp("xq", [128, 3, NPIX], BF16)
    # shared convs in fp8 DoubleRow: 6 pairs per cic; pair p<3 = taps
    # (p, p+3) i.e. (dy=0,dy=1) at dx=p; pair p>=3 = (tap 6+(p-3), zero)
    wsh0 = inp("wsh0", [128, 3, 6, 2, 128], FP8)
    wshm = inp("wshm", [128, 3, 6, 2, 128], FP8)
    # L0 gamma/beta weights, fp8, packed in DoubleRow tap pairs:
    # [ci, pair(5), plane(2), co_local]; pair 4 plane 1 is zeros
    wg = inp("wg", [128, 5, 2, COC], FP8)
    wbe = inp("wbe", [128, 5, 2, COC], FP8)
    wc0t = inp("wc0t", [128, 3, 8], BF16)
    wgbe1 = inp("wgbe1", [128, 9, 40], BF16)   # gamma @0:8, beta @32:40
    wgbe2 = inp("wgbe2", [128, 9, 48], BF16)   # gamma @0:16, beta @32:48
    # packed small tensors (see _pack_inputs for layouts)
    smallf = inp("smallf", [128, 41], F32)
    obc = inp("obc", [16, 384], F32)
    smallp = inp("smallp", [16, 7], F32)
    smallb = inp("smallb", [16, 17], BF16)

    out_t = nc.declare_dram_parameter("out", [1, NPIX], F32, isOutput=True)

    with tile.TileContext(nc) as tc:
        with (
            tc.tile_pool(name="work", bufs=1) as wpool,
            tc.tile_pool(name="dram", bufs=1, space="DRAM") as dpool,
            tc.tile_pool(name="ps", bufs=1, space="PSUM") as psB,
        ):
            # every pool costs ~2 serialized all-engine barrier groups in
            # the end-of-kernel teardown: keep exactly 3 pools and manage
            # buffer rotation manually via tags/slices
            cpool = wpool
            segpool = wpool
            wA = wpool

            # ---------- load constants / inputs into SBUF ----------
            def load(pool, ap, dtype=None, name=None):
                t = pool.tile(list(ap.shape), dtype or ap.dtype, tag=name)
                nc.sync.dma_start(out=t[:], in_=ap[:])
                return t

            seg_sb = load(wA, segbf, name="seg_sb")
            smallf_sb = load(cpool, smallf, name="smallf_sb")
            rtct_sb = load(wA, rtct, name="rtct_sb")
            rtct8_sb = load(wA, rtct8, name="rtct8_sb")
            fsems_sb = load(wA, fsems, name="fsems_sb")
            x_sb = load(cpool, xq, name="x_sb")
            obc_sb = load(cpool, obc, name="obc_sb")
            smallp_sb = load(cpool, smallp, name="smallp_sb")
            smallb_sb = load(cpool, smallb, name="smallb_sb")

            # views into the packed small tensors
            ones128f_sb = smallf_sb[:, 0:1]
            opg0_v = lambda m: smallf_sb[:, 1 + m:2 + m]
            bbe0a_v = lambda m: smallf_sb[:, 4 + m:5 + m]
            bsh0_v = smallf_sb[:, 7:8]
            bshm_v = smallf_sb[:, 8:9]
            segval_v = lambda s, n=128: smallf_sb[0:n, 9 + s:10 + s]
            negsegval_v = lambda s: smallf_sb[0:112, 25 + s:26 + s]
            obc0_v = obc_sb[0:G, 0:128]
            obc1_v = obc_sb[0:8, 128:256]
            obc2_v = obc_sb[0:16, 256:384]
            opg1_v, bbe1a_v = smallp_sb[0:8, 0:1], smallp_sb[0:8, 1:2]
            opg2_v, bbe2a_v = smallp_sb[0:16, 2:3], smallp_sb[0:16, 3:4]
            b0_v, b1_v, b2_v = (smallp_sb[0:8, 4:5], smallp_sb[0:16, 5:6],
                                smallp_sb[0:1, 6:7])
            wc1t_v = smallb_sb[0:8, 0:16]
            wc2t_v = smallb_sb[0:16, 16:17]

            # DRAM scratch
            crs_in = dpool.tile([SEGC, PAYW4], BF16)
            crs_out = dpool.tile([G, SEGC, PAYW4], BF16)
            crs2_in = dpool.tile([128, NPIX], BF16)
            crs2_out = dpool.tile([G, 128, NPIX], BF16)
            cc0_in = dpool.tile([8, NPIX], BF16)
            cc0_out = dpool.tile([8, NPIX], BF16)


            # Pre-allocate the 8 PSUM banks ONCE and rotate manually: every
            # pool.tile() call emits a TileRelease that drains serially in
            # the end-of-kernel teardown (~165ns each); phase A's tiles are
            # slices of the same 8 buffers. Dep tracking per buffer is
            # identical to pool rotation.
            bigA = [psB.tile([128, 512], F32, tag=f"bigA{i}", name=f"bigA{i}")
                    for i in range(4)]
            bigB = [psB.tile([128, 512], F32, tag=f"bigB{i}", name=f"bigB{i}")
                    for i in range(2)]
            accC = [psB.tile([128, 512], F32, tag=f"accC{i}", name=f"accC{i}")
                    for i in range(2)]
            _rot = {"A": 0, "B": 0, "C": 0}

            def take(which):
                lst = {"A": bigA, "B": bigB, "C": accC}[which]
                t = lst[_rot[which] % len(lst)]
                _rot[which] += 1
                return t

            # ---------- Phase A3: M matrices for my 16 segments ----------
            mall_ps = accC[0][0:32, :]
            for s in range(SEGC):
                oh = segpool.tile([112, 4, 448], FP8, tag="oh", bufs=3)
                nc.vector.tensor_scalar(oh[:], seg_sb[:],
                                        segval_v(s, 112), None,
                                        ALU.is_equal)
                # A^T[j,p] = sum_i oh[i,j] R[p,i]; the one-hot (0/1, exact
                # in fp8) is the DR lhsT with i-chunk pairs as the K
                # extension — halves the LDWEIGHTS-bound matmul count
                at_ps = bigA[s % 3][0:112, 0:128].rearrange(
                    "p (c q) -> p c q", c=4)
                for jb in range(4):
                    for pr in range(2):
                        nc.tensor.matmul(
                            at_ps[:, jb, :],
                            lhsT=oh[:, 2 * pr:2 * pr + 2,
                                    jb * 112:(jb + 1) * 112],
                            rhs=rtct8_sb[:, pr, :, :],
                            start=(pr == 0), stop=(pr == 1),
                            perf_mode=PM_DR)
                at_sb = segpool.tile([112, 4, 32], BF16, tag="atsb", bufs=3)
                nc.scalar.activation(at_sb[:], at_ps[:], AOT.Copy)
                for c in range(4):
                    nc.tensor.matmul(mall_ps[:, s * 32:(s + 1) * 32],
                                     lhsT=at_sb[:, c, :], rhs=rtct_sb[:, 4 + c, :],
                                     start=(c == 0), stop=(c == 3))

            # M [32(p), 16(s), 32(q)] -> payload rows [16(s), p*32+q]
            mall_bf = wpool.tile([32, SEGC, 32], BF16, tag="mall_bf")
            nc.vector.tensor_copy(
                mall_bf[:], mall_ps[:].rearrange("p (s q) -> p s q", s=SEGC))
            nc.sync.dma_start(
                out=crs_in[0:SEGC, CS:PAYW].rearrange(
                    "s (p q) -> p s q", p=32),
                in_=mall_bf[:])

            # ---------- Phase A1: LayerNorm0 partial stats from x chunk ----------
            xsum = wpool.tile([128, 1], F32, tag="xsum")
            nc.vector.tensor_reduce(xsum[:], x_sb[:], mybir.AxisListType.XY, ALU.add)
            xsq_scratch = wpool.tile([128, 3, NPIX], BF16, tag="sq_scratch")
            xsumsq = wpool.tile([128, 1], F32, tag="xsumsq")
            nc.scalar.activation(xsq_scratch[:], x_sb[:], AOT.Square,
                                 accum_out=xsumsq[:])
            stats2 = wpool.tile([128, 2], F32, tag="stats2")
            nc.vector.tensor_copy(stats2[:, 0:1], xsum[:])
            nc.vector.tensor_copy(stats2[:, 1:2], xsumsq[:])

            # cross-partition reduce of LN0 partial stats via ones-matmul
            stats1_ps = bigB[1][0:1, 0:2]
            nc.tensor.matmul(stats1_ps[:], lhsT=ones128f_sb, rhs=stats2[:],
                             start=True, stop=True)
            stats1_sb = wpool.tile([1, 2], F32, tag="stats1_sb")
            nc.scalar.activation(stats1_sb[:], stats1_ps[:], AOT.Copy)
            # ride the AllGather payload as raw bits (bf16 view of f32)
            nc.sync.dma_start(out=crs_in[0:1, PAYW:PAYW4],
                              in_=stats1_sb[:].bitcast(BF16))

            # ---------- Phase A2: segment averages for my 16 segments ----------
            ohsm = wpool.tile([128, SEGC, 8], BF16, tag="ohsm")
            for s in range(SEGC):
                nc.vector.tensor_scalar(ohsm[:, s, :], fsems_sb[:, :, 385],
                                        segval_v(s), None, ALU.is_equal)
            sums_ps = bigB[0][0:SEGC, 0:385]
            for c in range(8):
                nc.tensor.matmul(sums_ps[:], lhsT=ohsm[:, :, c],
                                 rhs=fsems_sb[:, c, 0:385],
                                 start=(c == 0), stop=(c == 7))
            sums_sb = wpool.tile([SEGC, 385], F32, tag="sums_sb")
            nc.scalar.activation(sums_sb[:], sums_ps[:], AOT.Copy)
            cnt_safe = wpool.tile([SEGC, 1], F32, tag="cnt_safe")
            nc.vector.tensor_scalar(cnt_safe[:], sums_sb[:, 384:385], 1.0, None,
                                    ALU.max)
            rec = wpool.tile([SEGC, 1], F32, tag="rec")
            nc.vector.reciprocal(rec[:], cnt_safe[:])
            mask = wpool.tile([SEGC, 1], F32, tag="mask")
            nc.vector.tensor_scalar(mask[:], sums_sb[:, 384:385], 0.5, None,
                                    ALU.is_gt)
            recm = wpool.tile([SEGC, 1], F32, tag="recm")
            nc.vector.tensor_mul(recm[:], rec[:], mask[:])
            avg_bf = wpool.tile([SEGC, CS], BF16, tag="avg_bf")
            nc.vector.tensor_scalar(avg_bf[:], sums_sb[:, 0:384], recm[:], None,
                                    ALU.mult)
            nc.sync.dma_start(out=crs_in[0:SEGC, 0:CS], in_=avg_bf[:])

            # ---------- C1: ONE small AllGather of (avg | M | stats) ----------
            nc.gpsimd.collective_compute(
                "AllGather", ALU.bypass,
                replica_groups=[[0, 1, 2, 3], [4, 5, 6, 7]],
                ins=[crs_in[:]], outs=[crs_out[:]],
            )

            wB = wpool

            # weights not needed until ~40us after the gather: load them
            # here so the pre-barrier input DMA stream stays short
            wshm_sb = load(wB, wshm, name="wshm_sb")
            wsh0_sb = load(wB, wsh0, name="wsh0_sb")
            wg_sb = load(wB, wg, name="wg_sb")
            wbe_sb = load(wB, wbe, name="wbe_sb")
            wc0t_sb = load(cpool, wc0t, name="wc0t_sb")
            wgbe1_sb = load(wB, wgbe1, name="wgbe1_sb")
            wgbe2_sb = load(wB, wgbe2, name="wgbe2_sb")

            # small PE filler after phase A keeps the HAM clock gate open a
            # bit longer into the gather window
            for i in range(8):
                fl = take("B")
                nc.tensor.matmul(fl[:], lhsT=x_sb[:, 0, 0:128],
                                 rhs=x_sb[:, 0, 0:512],
                                 start=True, stop=True)

            # ---------- Phase B0: unpack gather + LN0 scalars ----------
            avg_all = wB.tile([NSEG, CS], BF16, tag="avg_all")
            m_all = wB.tile([NSEG, NPIX], BF16, tag="m_all")
            nc.sync.dma_start(
                out=avg_all[:],
                in_=crs_out[:, :, 0:CS].rearrange("g s c -> (g s) c"))
            nc.sync.dma_start(
                out=m_all[:],
                in_=crs_out[:, :, CS:PAYW].rearrange("g s c -> (g s) c"))
            stats4 = wB.tile([G, 4], BF16, tag="stats4")
            nc.sync.dma_start(out=stats4[:], in_=crs_out[:, 0, PAYW:PAYW4])

            # reduce the 4 partial stats, divide by N (folded into the ones
            # block) AND broadcast to 128 partitions in one matmul
            st0_ps = take("C")[:, 0:2]
            nc.tensor.matmul(st0_ps[:], lhsT=obc0_v,
                             rhs=stats4[:].bitcast(F32), start=True, stop=True)
            # LN0 scalars live long (used across all m chunks): copy to SBUF
            st0_bc = wB.tile([128, 2], F32, tag="st0_bc")
            nc.scalar.activation(st0_bc[:], st0_ps[:], AOT.Copy)

            def ln_from_ms(ms_ap, nparts, scr):
                """ms_ap [nparts,2]=(mean, meansq); scr [nparts,4] F32
                scratch (musq|vare|lnv|istd) -> (mu AP, istd AP)."""
                musq, vare = scr[:, 0:1], scr[:, 1:2]
                lnv, istd = scr[:, 2:3], scr[:, 3:4]
                nc.scalar.activation(musq, ms_ap[:, 0:1], AOT.Square)
                nc.vector.scalar_tensor_tensor(vare, ms_ap[:, 1:2], EPS,
                                               musq, ALU.add, ALU.subtract)
                nc.scalar.activation(lnv, vare, AOT.Ln)
                nc.scalar.activation(istd, lnv, AOT.Exp, scale=-0.5)
                return ms_ap[:, 0:1], istd

            ln0scr = wB.tile([128, 4], F32, tag="ln0scr")
            mu0_bc, istd0_bc = ln_from_ms(st0_bc, 128, ln0scr[:])

            def pad_borders(t33):
                """zero only the 1-px border ring (interior is overwritten)."""
                nc.vector.memset(t33[:, 0:1, :], 0.0)
                nc.vector.memset(t33[:, 33:34, :], 0.0)
                nc.vector.memset(t33[:, 1:33, 0:1], 0.0)
                nc.vector.memset(t33[:, 1:33, 33:34], 0.0)

            # ---------- Phase B1: full sem_rs, padded, in SBUF (bf16) ----------
            semrs_pad = []
            for k in range(3):
                sp = wpool.tile([128, 34, 34], BF16, tag=f"semrs_pad{k}")
                pad_borders(sp)
                for h in range(2):
                    ps = take("A")
                    nc.tensor.matmul(ps[:],
                                     lhsT=avg_all[:, k * 128:(k + 1) * 128],
                                     rhs=m_all[:, h * 512:(h + 1) * 512],
                                     start=True, stop=True)
                    nc.scalar.activation(
                        sp[:, 1 + h * 16:17 + h * 16, 1:33],
                        ps[:].rearrange("c (r w) -> c r w", r=16), AOT.Copy)
                semrs_pad.append(sp)

            # ---------- shared 3x3 convs ----------
            # L0 is computed locally by every rank; ranks 0/1 compute the
            # full L1, ranks 2/3 the full L2, then an AllGather (hidden
            # under the L0 gamma/beta convs) distributes them.
            hsh_pad = [wpool.tile([128, 34, 34], BF16, tag=f"hsh_pad{L}",
                                  name=f"hsh_pad{L}")
                       for L in range(3)]

            def dr_win(sp, h, pr):
                """[128,2,16,32] view: plane 1 is plane 0 shifted one image
                row down (stride 48B). For pr>=3 plane 1 reads in-bounds
                garbage rows whose weights are zero."""
                dy0 = 0 if pr < 3 else 2
                dx = pr % 3
                ap = sp[:, h * 16 + dy0:h * 16 + dy0 + 16,
                        dx:dx + 32].copy()
                ap.ap.insert(1, (48, 2))
                return ap

            def sh_conv_half(ps, w_sb, h):
                for cic in range(3):
                    for pr in range(6):
                        nc.tensor.matmul(
                            ps[:].rearrange("c (r w) -> c r w", r=16),
                            lhsT=w_sb[:, cic, pr, :, :],
                            rhs=dr_win(semrs_pad[cic], h, pr),
                            start=(cic == 0 and pr == 0),
                            stop=(cic == 2 and pr == 5),
                            perf_mode=PM_DR)

            # my rank-pair's layer first, so the AllGather can start early
            hshm_flat = wpool.tile([128, NPIX], BF16, tag="hshm_flat")
            for h in range(2):
                ps = take("A")
                sh_conv_half(ps, wshm_sb, h)
                nc.scalar.activation(hshm_flat[:, h * 512:(h + 1) * 512],
                                     ps[:], AOT.Relu, bias=bshm_v)
            nc.sync.dma_start(out=crs2_in[:], in_=hshm_flat[:])
            nc.gpsimd.collective_compute(
                "AllGather", ALU.bypass,
                replica_groups=[[0, 1, 2, 3], [4, 5, 6, 7]],
                ins=[crs2_in[:]], outs=[crs2_out[:]],
            )

            # L0 locally
            pad_borders(hsh_pad[0])
            for h in range(2):
                ps = take("A")
                sh_conv_half(ps, wsh0_sb, h)
                nc.scalar.activation(
                    hsh_pad[0][:, 1 + h * 16:17 + h * 16, 1:33],
                    ps[:].rearrange("c (r w) -> c r w", r=16), AOT.Relu,
                    bias=bsh0_v)

            # ---------- Phase B5/B6: layer-0 gamma/beta convs + SPADE + c0 ----
            c0_ps = [take("C")[0:8, :] for h in range(2)]
            # fp8 interleaved window images (DoubleRow rhs): per half,
            # [pair(5), plane(2), 512]; pair p covers taps (2p, 2p+1),
            # pair 4 plane 1 is zeros
            im8 = [wpool.tile([128, 5, 2, 512], FP8, tag=f"im8_{h}",
                              name=f"im8_{h}") for h in range(2)]
            for h in range(2):
                for t in range(9):
                    dy, dx = t // 3, t % 3
                    nc.vector.tensor_copy(
                        im8[h][:, t // 2, t % 2, :].rearrange(
                            "c (r w) -> c r w", r=16),
                        _conv_windows(hsh_pad[0][:], 16, 32, dy, dx,
                                      row0=h * 16))
                nc.vector.memset(im8[h][:, 4, 1, :], 0.0)
            for m in range(3):
                xn_m = wpool.tile([128, NPIX], BF16, tag="xn", bufs=2)
                nc.vector.tensor_scalar(xn_m[:], x_sb[:, m, :], mu0_bc[:],
                                        istd0_bc[:], ALU.subtract, ALU.mult)
                for h in range(2):
                    gb_ps = {}
                    for name, w_sb in (("g", wg_sb), ("be", wbe_sb)):
                        ps = take("A") if name == "g" else take("B")
                        for pr in range(5):
                            nc.tensor.matmul(
                                ps[:],
                                lhsT=w_sb[:, pr, :, m * 128:(m + 1) * 128],
                                rhs=im8[h][:, pr, :, :],
                                start=(pr == 0), stop=(pr == 4),
                                perf_mode=PM_DR)
                        gb_ps[name] = ps
                    # spade: out = xn*(1+gamma+bg) + (beta+bbe)
                    hs = slice(h * 512, (h + 1) * 512)
                    u = wpool.tile([128, 512], BF16, tag="spade_u", bufs=2)
                    nc.vector.scalar_tensor_tensor(u[:], gb_ps["g"][:],
                                                   opg0_v(m),
                                                   xn_m[:, hs],
                                                   ALU.add, ALU.mult)
                    sp0 = wpool.tile([128, 512], BF16, tag="spade_o", bufs=2)
                    nc.vector.scalar_tensor_tensor(sp0[:], gb_ps["be"][:],
                                                   bbe0a_v(m), u[:],
                                                   ALU.add, ALU.add)
                    nc.tensor.matmul(c0_ps[h][:], lhsT=wc0t_sb[:, m, :],
                                     rhs=sp0[:], start=(m == 0), stop=(m == 2))

            c0p_sb = wpool.tile([8, NPIX], BF16, tag="f32buf")
            for h in range(2):
                nc.scalar.activation(c0p_sb[:, h * 512:(h + 1) * 512],
                                     c0_ps[h][:], AOT.Copy)
            nc.sync.dma_start(out=cc0_in[:], in_=c0p_sb[:])
            nc.gpsimd.collective_compute(
                "AllReduce", ALU.add,
                replica_groups=[[0, 1, 2, 3], [4, 5, 6, 7]],
                ins=[cc0_in[:]], outs=[cc0_out[:]],
            )

            # emitted after the collective trigger: unpack gathered sh layers
            # and prefire the L1/L2 conv matmuls so they overlap C2 on the PE
            hshg = [None, None]
            for L in (1, 2):
                hg = wpool.tile([128, NPIX], BF16, tag=f"hshg{L}",
                                name=f"hshg{L}")
                nc.sync.dma_start(out=hg[:], in_=crs2_out[0 if L == 1 else 2])
                hp = hsh_pad[L]
                pad_borders(hp)
                nc.vector.tensor_copy(
                    hp[:, 1:33, 1:33],
                    hg[:].rearrange("c (r w) -> c r w", r=32))
                hshg[L - 1] = hg

            def spade_conv_ps(nco, wgbe_sb, pad_img):
                pss = []
                for h in range(2):
                    p = take("A")[0:32 + nco, :]
                    for t in range(9):
                        dy, dx = t // 3, t % 3
                        nc.tensor.matmul(
                            p[:].rearrange("c (r w) -> c r w", r=16),
                            lhsT=wgbe_sb[:, t, :],
                            rhs=_conv_windows(pad_img[:], 16, 32, dy, dx,
                                              row0=h * 16),
                            start=(t == 0), stop=(t == 8))
                    pss.append(p)
                return pss

            l1_ps = spade_conv_ps(8, wgbe1_sb, hsh_pad[1])
            l2_ps = spade_conv_ps(16, wgbe2_sb, hsh_pad[2])


            # ---------- Phase B7: h1 + LN1 ----------
            # tail scratch is packed into few tiles (releases are per tile)
            spx_e = wpool.tile([16, NPIX], F32, tag="spx_e")
            lnpk = wpool.tile([16, 8], F32, tag="lnpk")
            st2pk = wpool.tile([16, 4], F32, tag="st2pk")
            # h1/h2/out overlap at partition 0: each is fully dead
            # (last read by t/Square) before the next one is written
            hpk = wpool.tile([16, NPIX], F32, tag="hpk")
            tpk = wpool.tile([16, NPIX], BF16, tag="tpk")
            upk = wpool.tile([16, NPIX], BF16, tag="upk")
            sppk = wpool.tile([16, NPIX], BF16, tag="sppk")

            def softplus_to(dst, src_aps, bias_ap, nparts, tag,
                            accum_out=None):
                """dst = ln(1 + exp(src + bias)); inputs here are small, so
                exp cannot overflow and both ACTs share one table set.
                src_aps: list of (ap, col0) slices of any width (PSUM/SBUF).
                accum_out, if given, receives sum(dst) for free."""
                for ap, col0 in src_aps:
                    w = ap.shape[-1]
                    nc.scalar.activation(spx_e[0:nparts, col0:col0 + w],
                                         ap, AOT.Exp, bias=bias_ap)
                nc.scalar.activation(dst[:], spx_e[0:nparts, :], AOT.Ln,
                                     bias=1.0, accum_out=accum_out)

            c0_sb = wpool.tile([8, NPIX], BF16, tag="f32buf2")
            nc.sync.dma_start(out=c0_sb[:], in_=cc0_out[:])
            st2_1 = st2pk[0:8, 0:2]
            h1_f32 = hpk[0:8, :]
            softplus_to(h1_f32, [(c0_sb[:, 0:NPIX], 0)],
                        b0_v, 8, "sp1", accum_out=st2_1[:, 0:1])

            def ln_small(h_f32, nparts, obc_sb, st2, scr):
                # Square's dst is never read (only accum_out matters):
                # dump it into spx_e, whose last reader has already run
                nc.scalar.activation(spx_e[0:nparts, :], h_f32[:], AOT.Square,
                                     accum_out=st2[:, 1:2])
                st_ps = take("C")[:, 0:2]
                nc.tensor.matmul(st_ps[:], lhsT=obc_sb[:],
                                 rhs=st2[:], start=True, stop=True)
                return ln_from_ms(st_ps[0:nparts, :], nparts, scr)

            # ---------- Phase B8: layers 1 and 2 (replicated) ----------
            def spade_small(h_f32, mu_ap, istd, nco, pss, opg_ap, tag):
                """spade with the LN affine re-associated off the critical
                path: t=(h-mu) and u=(gamma+opg)*t run on DVE while the ACT
                istd chain computes; only sp = u*istd + beta remains after
                istd. The beta conv BIAS is folded into the next stage's
                softplus bias host-side (W @ b_be)."""
                t_ = tpk[0:nco, :]
                nc.vector.tensor_scalar(t_[:], h_f32[:], mu_ap, None,
                                        ALU.subtract)
                sp_ = sppk[0:nco, :]
                for h in range(2):
                    hs = slice(h * 512, (h + 1) * 512)
                    nc.vector.scalar_tensor_tensor(upk[0:nco, hs],
                                                   pss[h][0:nco, :],
                                                   opg_ap, t_[:, hs],
                                                   ALU.add, ALU.mult)
                for h in range(2):
                    hs = slice(h * 512, (h + 1) * 512)
                    nc.vector.scalar_tensor_tensor(sp_[:, hs],
                                                   upk[0:nco, hs],
                                                   istd[:],
                                                   pss[h][32:32 + nco, :],
                                                   ALU.mult, ALU.add)
                return sp_

            mu1_bc, istd1_bc = ln_small(h1_f32, 8, obc1_v, st2_1,
                                        lnpk[0:8, 0:4])
            sp1 = spade_small(h1_f32, mu1_bc, istd1_bc, 8, l1_ps, opg1_v, "l1")
            c1_pss = []
            for h in range(2):
                c1_ps = take("C")[0:16, :]
                nc.tensor.matmul(c1_ps[:], lhsT=wc1t_v,
                                 rhs=sp1[:, h * 512:(h + 1) * 512],
                                 start=True, stop=True)
                c1_pss.append((c1_ps[:], h * 512))
            st2_2 = st2pk[0:16, 2:4]
            h2_f32 = hpk[0:16, :]
            softplus_to(h2_f32, c1_pss, b1_v, 16, "sp2",
                        accum_out=st2_2[:, 0:1])

            mu2_bc, istd2_bc = ln_small(h2_f32, 16, obc2_v, st2_2,
                                        lnpk[0:16, 4:8])
            sp2 = spade_small(h2_f32, mu2_bc, istd2_bc, 16, l2_ps, opg2_v,
                              "l2")
            c2_pss = []
            for h in range(2):
                c2_ps = take("C")[0:1, :]
                nc.tensor.matmul(c2_ps[:], lhsT=wc2t_v,
                                 rhs=sp2[:, h * 512:(h + 1) * 512],
                                 start=True, stop=True)
                c2_pss.append((c2_ps[:], h * 512))
            out_f32 = hpk[0:1, :]
            softplus_to(out_f32, c2_pss, b2_v, 1, "sp3")
            nc.sync.dma_start(out=out_t[:], in_=out_f32[:])

    nc.finalize()
    _split_sync_waits(nc)
    return nc


def _pack_inputs(inputs):
    f32 = np.float32
    R = _resize_matrix(HI, HP)      # [32, 448]
    C = _resize_matrix(WI, WP)      # [32, 448]
    rtct = np.zeros((112, 8, 32), f32)
    for c in range(4):
        rtct[:, c, :] = R[:, c * 112:(c + 1) * 112].T
        rtct[:, 4 + c, :] = C[:, c * 112:(c + 1) * 112].T
    rtct8 = np.zeros((112, 2, 2, 32), f32)
    for c in range(4):
        rtct8[:, c // 2, c % 2, :] = R[:, c * 112:(c + 1) * 112].T
    rtct8 = rtct8.astype(FP8_NP)
    rtct = rtct.astype(BF16_NP)

    segmap = inputs["segmap"]            # [2, 448, 448] int32
    f_sem = inputs["f_semantic"]         # [2, 384, 32, 32]
    x_main = inputs["x_main"]            # [2, 1536, 32, 32]
    rows = (np.arange(HP) * HI) // HP

    def tap_t(w):  # [co, ci, 3, 3] -> [ci, 9, co]
        return np.ascontiguousarray(w.transpose(1, 2, 3, 0).reshape(
            w.shape[1], 9, w.shape[0]))

    # shared conv layers, fp8 DR pairs: [ci, cic, pair, plane, co]
    def sh_pack(L):
        wt = tap_t(inputs[f"w_sh{L}"]).reshape(3, 128, 9, HM)  # cic,ci,t,co
        out = np.zeros((128, 3, 6, 2, HM), np.float32)
        for pr in range(3):
            out[:, :, pr, 0, :] = wt[:, :, pr, :].transpose(1, 0, 2)
            out[:, :, pr, 1, :] = wt[:, :, pr + 3, :].transpose(1, 0, 2)
            out[:, :, 3 + pr, 0, :] = wt[:, :, 6 + pr, :].transpose(1, 0, 2)
        return out.astype(FP8_NP)
    wsh_pk = [sh_pack(L) for L in range(3)]
    bsh_pk = [inputs[f"b_sh{L}"].reshape(128,) for L in range(3)]

    def gbe_pack(wg_, wbe_, nco):   # [128, 9, 32+nco], beta at col 32
        out = np.zeros((128, 9, 32 + nco), f32)
        out[:, :, 0:nco] = tap_t(wg_)
        out[:, :, 32:32 + nco] = tap_t(wbe_)
        return out.astype(BF16_NP)

    wgbe1 = gbe_pack(inputs["w_g1"], inputs["w_be1"], 8)
    wgbe2 = gbe_pack(inputs["w_g2"], inputs["w_be2"], 16)

    # smallp [16, 7]: opg1|bbe1a|opg2|bbe2a|b0|b1|b2
    smallp = np.zeros((16, 7), f32)
    smallp[0:8, 0] = 1.0 + inputs["b_g1"]
    smallp[0:8, 1] = inputs["b_be1"]
    smallp[0:16, 2] = 1.0 + inputs["b_g2"]
    smallp[0:16, 3] = inputs["b_be2"]
    smallp[0:8, 4] = inputs["bias0"]
    # beta conv biases of layers 1/2 are dropped from the spade output and
    # folded into the next pointwise conv's bias: W @ b_be is a constant
    smallp[0:16, 5] = (inputs["bias1"]
                       + inputs["w_c1"][:, :, 0, 0] @ inputs["b_be1"])
    smallp[0:1, 6] = (inputs["bias2"]
                      + inputs["w_c2"][:, :, 0, 0] @ inputs["b_be2"])

    # smallb [16, 17]: wc1t | wc2t
    smallb = np.zeros((16, 17), f32)
    smallb[0:8, 0:16] = inputs["w_c1"][:, :, 0, 0].T
    smallb[0:16, 16] = inputs["w_c2"][0, :, 0, 0]
    smallb = smallb.astype(BF16_NP)

    # obc [16, 384]: ones/N blocks for the LN reduce+broadcast matmuls
    obc = np.zeros((16, 384), f32)
    obc[0:4, 0:128] = 1.0 / LN0_N
    obc[0:8, 128:256] = 1.0 / LN1_N
    obc[0:16, 256:384] = 1.0 / LN2_N

    maps = []
    for cid in range(8):
        b, g = cid // G, cid % G
        d = {}
        seg = segmap[b].astype(f32)
        d["segbf"] = seg.reshape(4, 112, WI).transpose(1, 0, 2).astype(BF16_NP)
        d["rtct"] = rtct
        d["rtct8"] = rtct8
        seg_small = seg[rows[:, None], rows[None, :]].reshape(-1)   # [1024]
        segsm = seg_small.reshape(8, 128).T                          # [128, 8]
        fT = f_sem[b].reshape(CS, NPIX).T                           # [1024, 384]
        fTe = np.concatenate([fT, np.ones((NPIX, 1), f32)], 1)      # [1024, 385]
        fsems = np.zeros((128, 8, 386), f32)
        fsems[:, :, 0:385] = fTe.reshape(8, 128, 385).transpose(1, 0, 2)
        fsems[:, :, 385] = segsm
        d["fsems"] = fsems.astype(BF16_NP)
        xc = x_main[b, g * COC:(g + 1) * COC].reshape(COC, NPIX)
        d["xq"] = np.ascontiguousarray(
            xc.reshape(3, 128, NPIX).transpose(1, 0, 2)).astype(BF16_NP)
        d["wsh0"] = wsh_pk[0]
        mine = 1 if g < 2 else 2
        d["wshm"] = wsh_pk[mine]
        cosl = slice(g * COC, (g + 1) * COC)
        def dr_pack(w):  # [128,9,384] -> [128, 5, 2, 384] fp8, tap pairs
            wt = tap_t(w)
            out = np.zeros((128, 5, 2, COC), np.float32)
            for t in range(9):
                out[:, t // 2, t % 2, :] = wt[:, t, :]
            return out.astype(FP8_NP)

        d["wg"] = dr_pack(inputs["w_g0"][cosl])
        d["wbe"] = dr_pack(inputs["w_be0"][cosl])
        wc0 = inputs["w_c0"][:, :, 0, 0]                           # [8, 1536]
        d["wc0t"] = np.ascontiguousarray(
            wc0[:, cosl].T.reshape(3, 128, 8).transpose(1, 0, 2)).astype(BF16_NP)
        d["wgbe1"] = wgbe1
        d["wgbe2"] = wgbe2
        # smallf [128, 41]: ones | opg0(3) | bbe0a(3) | bsh0 | bshm |
        # segval(16) | -segval(16)
        smallf = np.zeros((128, 41), f32)
        smallf[:, 0] = 1.0
        smallf[:, 1:4] = (1.0 + inputs["b_g0"][cosl]).reshape(3, 128).T
        smallf[:, 4:7] = inputs["b_be0"][cosl].reshape(3, 128).T
        smallf[:, 7] = bsh_pk[0]
        smallf[:, 8] = bsh_pk[mine]
        segv = (np.arange(SEGC, dtype=f32) + SEGC * g)[None, :]
        smallf[:, 9:25] = segv
        smallf[:, 25:41] = -segv
        d["smallf"] = smallf
        d["obc"] = obc
        d["smallp"] = smallp
        d["smallb"] = smallb
        maps.append(d)
    return maps


def kernel(**inputs):
    if "nc" not in _NC_CACHE:
        _NC_CACHE["nc"] = _build_nc()
    nc = _NC_CACHE["nc"]
    in_maps = _pack_inputs(inputs)
    res = run_bass_kernel_spmd(nc, in_maps, list(range(8)))
    out = np.zeros((B, 1, HP, WP), np.float32)
    out[0, 0] = res.results[0]["out"].reshape(HP, WP)
    out[1, 0] = res.results[4]["out"].reshape(HP, WP)
    return out


if __name__ == "__main__":
    nc = _build_nc()
    print("built OK; instructions:",
          sum(len(b.instructions) for f in nc.m.functions for b in f.blocks))

